# revision 1
# baseline (speedup 1.0000x reference)
"""Trainium2 Bass kernel for bidirectional ActionLSTM.

Full inputs in, full output out. Internally: data-parallel over batch
(8 NeuronCores x 256 batch rows), LSTM weights replicated.

Device program (per core, transposed layout: hidden on partitions,
batch on the free dim):
  - fc_in is folded into the LSTM input weights on the host:
        W_x = w_ih @ fc_in_w  [512, 68],  bias = w_ih@fc_in_b + b_ih + b_hh
    and the bias is folded in as an extra all-ones input row (K=69).
  - Gate order permuted to [i, f, o, g]. tanh is computed via the
    identity tanh(z) = 2*sigmoid(2z) - 1, with the 2z folded into the
    g-gate weight rows, so ONE sigmoid activation op covers all gates.
  - h is stored as h/2 ("h_half"); the 2x is folded into the recurrent
    and pooling weights. tanh(c) likewise becomes sigmoid(2c) via the
    activation's free scale.
  - Mean-pool over time + fc_out are folded into a per-step accumulating
    matmul into PSUM: pacc += (2/T * Wo_dir) @ h_half_t.
Per (step, dir): 9 matmuls (4 x-side K=69 + 4 recurrent K=128 + 1 pool),
1 big sigmoid [128,1024], 1 sigmoid(2c) [128,256] on ACT, 3
scalar_tensor_tensor ops on DVE, 1 tensor_tensor on GpSimd.
"""

import os
import numpy as np
from contextlib import ExitStack

INPUT, HID, NCLS = 68, 128, 3
B, T = 2048, 128
NCORES = 8
BL = B // NCORES          # 256 batch rows per core
KX = INPUT + 1            # 69 (ones row folds bias in)
G4 = 4 * HID              # 512

# matmul operand dtype: "f32r" (full fp32 storage, fast tensor-engine mode)
# or "bf16"
MM_DTYPE = os.environ.get("LSTM_MM_DTYPE", "bf16")
USE_FILLER = os.environ.get("LSTM_FILLER", "1") == "1"
N_FILLER = int(os.environ.get("LSTM_N_FILLER", "3"))
CELL_BF16 = os.environ.get("LSTM_CELL_BF16", "0") == "1"

_CACHE = {}


def _build_program():
    import concourse.bass as bass
    import concourse.tile as tile
    from concourse import bacc, mybir

    f32 = mybir.dt.float32
    AF = mybir.ActivationFunctionType
    OP = mybir.AluOpType
    use_bf16 = MM_DTYPE == "bf16"
    # dtype for tensors consumed by the tensor engine (x, weights, h):
    # bf16 runs the matmul at full rate (fp32r measured at 1/4 rate on HW);
    # gate accumulation stays fp32 in PSUM, s/c stay fp32 on DVE/ACT.
    sb_dt = mybir.dt.bfloat16 if use_bf16 else mybir.dt.float32r

    cell_dt = mybir.dt.bfloat16 if CELL_BF16 else f32

    def R(ap):
        return ap

    nc = bacc.Bacc("TRN2", target_bir_lowering=False, debug=False,
                   num_devices=NCORES)

    xin = nc.dram_tensor("xin", [KX, T * BL], sb_dt, kind="ExternalInput").ap()
    wx = {}
    wu = {}
    for d in "fb":
        wx[d] = nc.dram_tensor(f"wx_{d}", [KX, G4], sb_dt,
                               kind="ExternalInput").ap()
        wu[d] = nc.dram_tensor(f"wu_{d}", [HID, G4], sb_dt,
                               kind="ExternalInput").ap()
    out = nc.dram_tensor("out", [2 * HID, BL], f32, kind="ExternalOutput").ap()

    with tile.TileContext(nc) as tc, ExitStack() as ctx:
        const = ctx.enter_context(tc.tile_pool(name="const", bufs=1))
        X = const.tile([KX, T * BL], sb_dt, tag="X")
        # split the big input DMA into chunks so it spreads across DMA
        # queues and so early timesteps unblock compute quickly; issue
        # from both ends since the bwd direction consumes t=T-1 first.
        NCHUNK = 16
        CW = T * BL // NCHUNK
        order = []
        for i in range(NCHUNK // 2):
            order += [NCHUNK - 1 - i, i]
        for ci in order:
            nc.sync.dma_start(X[:, ci * CW:(ci + 1) * CW],
                              xin[:, ci * CW:(ci + 1) * CW])

        WX = {}
        WU = {}
        for d in "fb":
            WX[d] = const.tile([KX, G4], sb_dt, tag=f"wx{d}", name=f"WX{d}")
            nc.sync.dma_start(WX[d][:], wx[d][:])
            WU[d] = const.tile([HID, G4], sb_dt, tag=f"wu{d}", name=f"WU{d}")
            nc.sync.dma_start(WU[d][:], wu[d][:])

        hpool = ctx.enter_context(tc.tile_pool(name="h", bufs=4))
        cpool = ctx.enter_context(tc.tile_pool(name="c", bufs=4))
        spool = ctx.enter_context(tc.tile_pool(name="s", bufs=3))
        scpool = ctx.enter_context(tc.tile_pool(name="sc", bufs=3))
        mpool = ctx.enter_context(tc.tile_pool(name="m1h", bufs=3))
        tpool = ctx.enter_context(tc.tile_pool(name="tt", bufs=3))
        gpsum = ctx.enter_context(tc.tile_pool(name="gates", bufs=1,
                                               space="PSUM"))

        h = {}
        c = {}
        hsum = {}
        gates = {}
        for d in "fb":
            h[d] = hpool.tile([HID, BL], sb_dt, tag=f"h{d}", name=f"h0{d}")
            nc.vector.memset(h[d][:].bitcast(f32) if not use_bf16 else h[d][:],
                             0.0)
            c[d] = cpool.tile([HID, BL], cell_dt, tag=f"c{d}", name=f"c0{d}")
            nc.vector.memset(c[d][:], 0.0)
            hsum[d] = const.tile([HID, BL], f32, tag=f"hs{d}", name=f"hsum{d}")
            nc.vector.memset(hsum[d][:], 0.0)

        def emit_x_window(t):
            # x-side window matmuls for steps (t, t+1), both dirs: one N=512
            # matmul per gate legally "starts" (zeroes) its whole PSUM bank
            # (PSUM pending-zero works on whole 2KB per-partition regions, so
            # per-bank a single start matmul must precede all accumulates),
            # and runs during the previous step's sigmoid/DVE phase.
            for d in ("f", "b"):
                g = gpsum.tile([HID, 4, 2 * BL], f32, tag=f"g{d}",
                               name=f"g_{d}_{t}")
                gates[d] = g
                if d == "f":
                    xw = X[:, t * BL:(t + 2) * BL]
                else:
                    xw = X[:, (T - 2 - t) * BL:(T - t) * BL]
                for gi in range(4):
                    nc.tensor.matmul(g[:, gi, :],
                                     R(WX[d][:, gi * HID:(gi + 1) * HID]),
                                     R(xw), start=True, stop=False,
                                     skip_group_check=True)

        emit_x_window(0)
        for t in range(T):
            # phase 1: recurrent matmuls + big sigmoid, per dir — the two
            # dirs' sigmoids sit adjacent in the ACT FIFO so ACT works dir b
            # while dir f's DVE chain runs (anti-phase staggering)
            sd = {}
            for d in ("f", "b"):
                # recurrent matmuls accumulate into this step's column half
                # of each gate bank; the two dirs' sigmoids sit adjacent in
                # the ACT FIFO so ACT serves dir b while dir f's DVE chain
                # runs (anti-phase staggering of the two chains)
                g = gates[d]
                cs = (t % 2 if d == "f" else 1 - (t % 2)) * BL
                for gi in range(4):
                    nc.tensor.matmul(g[:, gi, cs:cs + BL],
                                     R(WU[d][:, gi * HID:(gi + 1) * HID]),
                                     R(h[d][:]), start=False,
                                     stop=(t % 2 == 1),
                                     skip_group_check=True)
                s = spool.tile([HID, 4, BL], cell_dt, tag=f"s{d}", name=f"s{d}{t}")
                nc.scalar.activation(s[:], g[:, :, cs:cs + BL], AF.Sigmoid)
                sd[d] = s
            # phase 2: cell updates, stage-interleaved across dirs
            ttd = {}
            md = {}
            cn = {}
            scd = {}
            for d in ("f", "b"):
                s = sd[d]
                tt = tpool.tile([HID, BL], cell_dt, tag=f"tt{d}", name=f"tt{d}{t}")
                nc.vector.tensor_tensor(tt[:], s[:, 1, :], c[d][:], op=OP.mult)
                m1h = mpool.tile([HID, BL], cell_dt, tag=f"m{d}", name=f"m{d}{t}")
                nc.vector.scalar_tensor_tensor(m1h[:], s[:, 3, :], 0.5,
                                               s[:, 0, :],
                                               op0=OP.subtract, op1=OP.mult)
                ttd[d] = tt
                md[d] = m1h
            for d in ("f", "b"):
                c_new = cpool.tile([HID, BL], cell_dt, tag=f"c{d}", name=f"c{d}{t}")
                nc.vector.tensor_tensor(c_new[:], md[d][:], ttd[d][:],
                                        op=OP.add)
                sc = scpool.tile([HID, BL], cell_dt, tag=f"sc{d}", name=f"sc{d}{t}")
                nc.scalar.activation(sc[:], c_new[:], AF.Sigmoid, scale=4.0)
                cn[d] = c_new
                scd[d] = sc
                # dir f's h-STT emitted BEFORE dir b's C/sc ops: in the
                # in-order DVE FIFO this lets h_f issue as soon as sc_f is
                # ready instead of queueing behind dir b's cell ops, so the
                # next step's U matmuls (gated on h) start earlier.
                if d == "f":
                    h_new = hpool.tile([HID, BL], sb_dt, tag="hf",
                                       name=f"hf{t}")
                    nc.vector.scalar_tensor_tensor(h_new[:], scd["f"][:], 0.5,
                                                   sd["f"][:, 2, :],
                                                   op0=OP.subtract,
                                                   op1=OP.mult)
                    nc.gpsimd.tensor_tensor(hsum["f"][:], hsum["f"][:],
                                            h_new[:], op=OP.add)
                    h["f"] = h_new
                    c["f"] = cn["f"]
            h_new = hpool.tile([HID, BL], sb_dt, tag="hb", name=f"hb{t}")
            nc.vector.scalar_tensor_tensor(h_new[:], scd["b"][:], 0.5,
                                           sd["b"][:, 2, :],
                                           op0=OP.subtract, op1=OP.mult)
            nc.gpsimd.tensor_tensor(hsum["b"][:], hsum["b"][:], h_new[:],
                                    op=OP.add)
            h["b"] = h_new
            c["b"] = cn["b"]
            if t % 2 == 1 and t < T - 1:
                emit_x_window(t + 1)

        nc.sync.dma_start(out[0:HID, :], hsum["f"][:])
        nc.sync.dma_start(out[HID:2 * HID, :], hsum["b"][:])

    nc.compile()
    return nc


def _prep_weights(w_ih, w_hh, b_ih, b_hh, fc_in_w, fc_in_b):
    Wx = w_ih.astype(np.float64) @ fc_in_w.astype(np.float64)   # [512, 68]
    bias = w_ih.astype(np.float64) @ fc_in_b.astype(np.float64) \
        + b_ih.astype(np.float64) + b_hh.astype(np.float64)
    perm = np.concatenate([np.arange(0, 128), np.arange(128, 256),
                           np.arange(384, 512), np.arange(256, 384)])
    Wx = Wx[perm]
    U = w_hh.astype(np.float64)[perm]
    bias = bias[perm]
    srow = np.ones((512, 1), np.float64)
    srow[384:] = 2.0
    Wx_aug = np.concatenate([Wx, bias[:, None]], axis=1)        # [512, 69]
    lhsT_x = np.ascontiguousarray((srow * Wx_aug).T)            # [69, 512]
    lhsT_U = np.ascontiguousarray((srow * U * 2.0).T)           # [128, 512]
    return lhsT_x, lhsT_U


def kernel(x, fc_in_w, fc_in_b, w_ih_f, w_hh_f, b_ih_f, b_hh_f,
           w_ih_b, w_hh_b, b_ih_b, b_hh_b, fc_out_w, fc_out_b,
           _want_trace=False):
    from concourse import bass_utils

    np_dt = np.float32
    if MM_DTYPE == "bf16":
        import ml_dtypes
        np_dt = ml_dtypes.bfloat16

    if "nc" not in _CACHE:
        _CACHE["nc"] = _build_program()
    nc = _CACHE["nc"]

    lx_f, lU_f = _prep_weights(w_ih_f, w_hh_f, b_ih_f, b_hh_f,
                               fc_in_w, fc_in_b)
    lx_b, lU_b = _prep_weights(w_ih_b, w_hh_b, b_ih_b, b_hh_b,
                               fc_in_w, fc_in_b)
    shared = {
        "wx_f": lx_f.astype(np_dt), "wu_f": lU_f.astype(np_dt),
        "wx_b": lx_b.astype(np_dt), "wu_b": lU_b.astype(np_dt),
    }
    wo_f = fc_out_w[:, :HID].astype(np.float64)   # [3, 128]
    wo_b = fc_out_w[:, HID:].astype(np.float64)
    in_maps = []
    for cidx in range(NCORES):
        xs = x[cidx * BL:(cidx + 1) * BL]                    # [BL, T, 68]
        xT = np.ascontiguousarray(xs.transpose(2, 1, 0))     # [68, T, BL]
        x_aug = np.concatenate(
            [xT, np.ones((1, T, BL), np.float32)], axis=0)   # [69, T, BL]
        in_maps.append({"xin": x_aug.reshape(KX, T * BL).astype(np_dt),
                        **shared})

    res = bass_utils.run_bass_kernel_spmd(
        nc, in_maps, core_ids=list(range(NCORES)), trace=_want_trace)
    outs = []
    for cidx in range(NCORES):
        o = res.results[cidx]["out"].astype(np.float64)       # [2H, BL]
        pool = (2.0 / T) * (wo_f @ o[0:HID] + wo_b @ o[HID:])  # [3, BL]
        out_core = pool.T + fc_out_b                          # [BL, 3]
        outs.append(out_core)
    full = np.concatenate(outs, axis=0).astype(np.float32)
    if _want_trace:
        _CACHE["last_result"] = res
    return full



# revision 2
# speedup vs baseline: 1.1039x; 1.1039x over previous
"""Trainium2 Bass kernel for bidirectional ActionLSTM.

Full inputs in, full output out. Internally: data-parallel over batch
(8 NeuronCores x 256 batch rows), LSTM weights replicated.

Device program (per core, transposed layout: hidden on partitions,
batch on the free dim):
  - fc_in is folded into the LSTM input weights on the host:
        W_x = w_ih @ fc_in_w  [512, 68],  bias = w_ih@fc_in_b + b_ih + b_hh
    and the bias is folded in as an extra all-ones input row (K=69).
  - Gate order permuted to [i, f, o, g]. tanh is computed via the
    identity tanh(z) = 2*sigmoid(2z) - 1, with the 2z folded into the
    g-gate weight rows, so ONE sigmoid activation op covers all gates.
  - h is stored as h/2 ("h_half"); the 2x is folded into the recurrent
    and pooling weights. tanh(c) likewise becomes sigmoid(2c) via the
    activation's free scale.
  - Mean-pool over time + fc_out: hsum accumulated on the Pool engine,
    fc_out applied on the host.

v2 restructure vs v1:
  - Per-(dir, step) PSUM gate tiles [128, 4, BL] (2 banks), double
    buffered (bufs=2) -> 8 banks total; removes the v1 2-step-window
    full-PSUM barrier that phase-locked the two directions.
  - Cell/elementwise path in bf16 end-to-end (DVE 2x mode).
  - Emission order tuned per engine FIFO: ACT sees
    [SIG_f, SIG_b, sc_f, sc_b] per step (no head-of-line stall), PE
    sees [U_f, X_f(t+1), U_b, X_b(t+1)] so x-side matmuls fill the
    PE gap while waiting on the other dir's h.
  - U matmuls skipped at t=0 (h0 == 0); x-side matmuls carry the
    full gates at t=0.
  - Optional (LSTM_X_FP8=1): x-side matmuls in fp8e4 DoubleRow mode
    (K=69 padded to 128, split into 2 k-tiles of 64) at 2x column
    rate; recurrent path stays bf16.
"""

import os
import numpy as np
from contextlib import ExitStack

INPUT, HID, NCLS = 68, 128, 3
B, T = 2048, 128
NCORES = 8
BL = B // NCORES          # 256 batch rows per core
KX = INPUT + 1            # 69 (ones row folds bias in)
G4 = 4 * HID              # 512

X_FP8 = os.environ.get("LSTM_X_FP8", "0") == "1"
CELL_F32 = os.environ.get("LSTM_CELL_F32", "0") == "1"

_CACHE = {}


def _build_program():
    import concourse.bass as bass
    import concourse.tile as tile
    from concourse import bacc, mybir

    f32 = mybir.dt.float32
    bf16 = mybir.dt.bfloat16
    fp8 = mybir.dt.float8e4
    AF = mybir.ActivationFunctionType
    OP = mybir.AluOpType
    DR = mybir.MatmulPerfMode.DoubleRow

    cell_dt = f32 if CELL_F32 else bf16

    nc = bacc.Bacc("TRN2", target_bir_lowering=False, debug=False,
                   num_devices=NCORES)

    if X_FP8:
        xin = nc.dram_tensor("xin", [64, 2, T * BL], fp8,
                             kind="ExternalInput").ap()
    else:
        xin = nc.dram_tensor("xin", [KX, T * BL], bf16,
                             kind="ExternalInput").ap()
    wx = {}
    wu = {}
    for d in "fb":
        if X_FP8:
            wx[d] = nc.dram_tensor(f"wx_{d}", [64, 2, G4], fp8,
                                   kind="ExternalInput").ap()
        else:
            wx[d] = nc.dram_tensor(f"wx_{d}", [KX, G4], bf16,
                                   kind="ExternalInput").ap()
        wu[d] = nc.dram_tensor(f"wu_{d}", [HID, G4], bf16,
                               kind="ExternalInput").ap()
    out = nc.dram_tensor("out", [2 * HID, BL], f32, kind="ExternalOutput").ap()

    with tile.TileContext(nc) as tc, ExitStack() as ctx:
        const = ctx.enter_context(tc.tile_pool(name="const", bufs=1))
        if X_FP8:
            X = const.tile([64, 2, T * BL], fp8, tag="X")
        else:
            X = const.tile([KX, T * BL], bf16, tag="X")
        # split the big input DMA into chunks so it spreads across DMA
        # queues and so early timesteps unblock compute quickly; issue
        # from both ends since the bwd direction consumes t=T-1 first.
        NCHUNK = 16
        CW = T * BL // NCHUNK
        order = []
        for i in range(NCHUNK // 2):
            order += [NCHUNK - 1 - i, i]
        for ci in order:
            if X_FP8:
                nc.sync.dma_start(X[:, :, ci * CW:(ci + 1) * CW],
                                  xin[:, :, ci * CW:(ci + 1) * CW])
            else:
                nc.sync.dma_start(X[:, ci * CW:(ci + 1) * CW],
                                  xin[:, ci * CW:(ci + 1) * CW])

        WX = {}
        WU = {}
        for d in "fb":
            if X_FP8:
                WX[d] = const.tile([64, 2, G4], fp8, tag=f"wx{d}",
                                   name=f"WX{d}")
            else:
                WX[d] = const.tile([KX, G4], bf16, tag=f"wx{d}",
                                   name=f"WX{d}")
            nc.sync.dma_start(WX[d][:], wx[d][:])
            WU[d] = const.tile([HID, G4], bf16, tag=f"wu{d}", name=f"WU{d}")
            nc.sync.dma_start(WU[d][:], wu[d][:])

        hpool = ctx.enter_context(tc.tile_pool(name="h", bufs=4))
        cpool = ctx.enter_context(tc.tile_pool(name="c", bufs=4))
        spool = ctx.enter_context(tc.tile_pool(name="s", bufs=3))
        scpool = ctx.enter_context(tc.tile_pool(name="sc", bufs=3))
        mpool = ctx.enter_context(tc.tile_pool(name="m1h", bufs=3))
        tpool = ctx.enter_context(tc.tile_pool(name="tt", bufs=3))
        gpsum = ctx.enter_context(tc.tile_pool(name="gates", bufs=2,
                                               space="PSUM"))

        h = {}
        c = {}
        hsum = {}
        for d in "fb":
            c[d] = cpool.tile([HID, BL], cell_dt, tag=f"c{d}", name=f"c0{d}")
            nc.vector.memset(c[d][:], 0.0)
            hsum[d] = const.tile([HID, BL], f32, tag=f"hs{d}", name=f"hsum{d}")
            nc.vector.memset(hsum[d][:], 0.0)

        g_cur = {}

        def emit_x(d, t):
            # x-side matmuls for step t into a fresh (ring) PSUM tile.
            # Each gate pair shares a 2KB PSUM bank: the even gate's
            # start=True matmul pending-zeroes the whole bank, the odd
            # gate accumulates into its (still pending-zero) half.
            g = gpsum.tile([HID, 4, BL], f32, tag=f"g{d}", name=f"g_{d}_{t}")
            if d == "f":
                ts = t
            else:
                ts = T - 1 - t
            for gi in range(4):
                if X_FP8:
                    nc.tensor.matmul(g[:, gi, :],
                                     WX[d][:, :, gi * HID:(gi + 1) * HID],
                                     X[:, :, ts * BL:(ts + 1) * BL],
                                     start=(gi % 2 == 0),
                                     stop=(t == 0 and gi % 2 == 1),
                                     perf_mode=DR,
                                     skip_group_check=True)
                else:
                    nc.tensor.matmul(g[:, gi, :],
                                     WX[d][:, gi * HID:(gi + 1) * HID],
                                     X[:, ts * BL:(ts + 1) * BL],
                                     start=(gi % 2 == 0),
                                     stop=(t == 0 and gi % 2 == 1),
                                     skip_group_check=True)
            return g

        for d in ("f", "b"):
            g_cur[d] = emit_x(d, 0)

        for t in range(T):
            sd = {}
            for d in ("f", "b"):
                g = g_cur[d]
                if t > 0:
                    for gi in range(4):
                        nc.tensor.matmul(g[:, gi, :],
                                         WU[d][:, gi * HID:(gi + 1) * HID],
                                         h[d][:], start=False,
                                         stop=(gi % 2 == 1),
                                         skip_group_check=True)
                if t + 1 < T:
                    g_cur[d] = emit_x(d, t + 1)
                s = spool.tile([HID, 4, BL], cell_dt, tag=f"s{d}",
                               name=f"s{d}{t}")
                nc.scalar.activation(s[:], g[:], AF.Sigmoid)
                sd[d] = s
            # cell updates; DVE FIFO: tt_f, m1h_f, c_f, tt_b, m1h_b, c_b
            # so dir f's c lands as early as possible (its sc queues on
            # ACT right behind dir b's big sigmoid).
            cn = {}
            for d in ("f", "b"):
                s = sd[d]
                tt = tpool.tile([HID, BL], cell_dt, tag=f"tt{d}",
                                name=f"tt{d}{t}")
                nc.vector.tensor_tensor(tt[:], s[:, 1, :], c[d][:],
                                        op=OP.mult)
                m1h = mpool.tile([HID, BL], cell_dt, tag=f"m{d}",
                                 name=f"m{d}{t}")
                nc.vector.scalar_tensor_tensor(m1h[:], s[:, 3, :], 0.5,
                                               s[:, 0, :],
                                               op0=OP.subtract, op1=OP.mult)
                c_new = cpool.tile([HID, BL], cell_dt, tag=f"c{d}",
                                   name=f"c{d}{t}")
                nc.vector.tensor_tensor(c_new[:], m1h[:], tt[:], op=OP.add)
                cn[d] = c_new
            scd = {}
            for d in ("f", "b"):
                sc = scpool.tile([HID, BL], cell_dt, tag=f"sc{d}",
                                 name=f"sc{d}{t}")
                nc.scalar.activation(sc[:], cn[d][:], AF.Sigmoid, scale=4.0)
                scd[d] = sc
            for d in ("f", "b"):
                h_new = hpool.tile([HID, BL], bf16, tag=f"h{d}",
                                   name=f"h{d}{t}")
                nc.vector.scalar_tensor_tensor(h_new[:], scd[d][:], 0.5,
                                               sd[d][:, 2, :],
                                               op0=OP.subtract, op1=OP.mult)
                h[d] = h_new
                c[d] = cn[d]
            for d in ("f", "b"):
                nc.gpsimd.tensor_tensor(hsum[d][:], hsum[d][:], h[d][:],
                                        op=OP.add)

        nc.sync.dma_start(out[0:HID, :], hsum["f"][:])
        nc.sync.dma_start(out[HID:2 * HID, :], hsum["b"][:])

    nc.compile()
    return nc


def _prep_weights(w_ih, w_hh, b_ih, b_hh, fc_in_w, fc_in_b):
    Wx = w_ih.astype(np.float64) @ fc_in_w.astype(np.float64)   # [512, 68]
    bias = w_ih.astype(np.float64) @ fc_in_b.astype(np.float64) \
        + b_ih.astype(np.float64) + b_hh.astype(np.float64)
    perm = np.concatenate([np.arange(0, 128), np.arange(128, 256),
                           np.arange(384, 512), np.arange(256, 384)])
    Wx = Wx[perm]
    U = w_hh.astype(np.float64)[perm]
    bias = bias[perm]
    srow = np.ones((512, 1), np.float64)
    srow[384:] = 2.0
    Wx_aug = np.concatenate([Wx, bias[:, None]], axis=1)        # [512, 69]
    lhsT_x = np.ascontiguousarray((srow * Wx_aug).T)            # [69, 512]
    lhsT_U = np.ascontiguousarray((srow * U * 2.0).T)           # [128, 512]
    return lhsT_x, lhsT_U


def _split_k64(a):
    # [69, N] -> [64, 2, N]: pad K to 128, k-tile q covers rows q*64..q*64+63
    n = a.shape[1]
    ap = np.zeros((128, n), np.float64)
    ap[:a.shape[0]] = a
    return np.ascontiguousarray(ap.reshape(2, 64, n).transpose(1, 0, 2))


def kernel(x, fc_in_w, fc_in_b, w_ih_f, w_hh_f, b_ih_f, b_hh_f,
           w_ih_b, w_hh_b, b_ih_b, b_hh_b, fc_out_w, fc_out_b,
           _want_trace=False):
    from concourse import bass_utils
    import ml_dtypes

    bf16 = ml_dtypes.bfloat16
    fp8 = ml_dtypes.float8_e4m3fn

    if "nc" not in _CACHE:
        _CACHE["nc"] = _build_program()
    nc = _CACHE["nc"]

    lx_f, lU_f = _prep_weights(w_ih_f, w_hh_f, b_ih_f, b_hh_f,
                               fc_in_w, fc_in_b)
    lx_b, lU_b = _prep_weights(w_ih_b, w_hh_b, b_ih_b, b_hh_b,
                               fc_in_w, fc_in_b)
    if X_FP8:
        shared = {
            "wx_f": _split_k64(lx_f).astype(fp8),
            "wx_b": _split_k64(lx_b).astype(fp8),
        }
    else:
        shared = {"wx_f": lx_f.astype(bf16), "wx_b": lx_b.astype(bf16)}
    shared["wu_f"] = lU_f.astype(bf16)
    shared["wu_b"] = lU_b.astype(bf16)

    wo_f = fc_out_w[:, :HID].astype(np.float64)   # [3, 128]
    wo_b = fc_out_w[:, HID:].astype(np.float64)
    in_maps = []
    for cidx in range(NCORES):
        xs = x[cidx * BL:(cidx + 1) * BL]                    # [BL, T, 68]
        xT = np.ascontiguousarray(xs.transpose(2, 1, 0))     # [68, T, BL]
        x_aug = np.concatenate(
            [xT, np.ones((1, T, BL), np.float32)], axis=0)   # [69, T, BL]
        x_aug = x_aug.reshape(KX, T * BL)
        if X_FP8:
            xm = _split_k64(x_aug).astype(fp8)               # [64, 2, T*BL]
        else:
            xm = x_aug.astype(bf16)
        in_maps.append({"xin": xm, **shared})

    res = bass_utils.run_bass_kernel_spmd(
        nc, in_maps, core_ids=list(range(NCORES)), trace=_want_trace)
    outs = []
    for cidx in range(NCORES):
        o = res.results[cidx]["out"].astype(np.float64)       # [2H, BL]
        pool = (2.0 / T) * (wo_f @ o[0:HID] + wo_b @ o[HID:])  # [3, BL]
        out_core = pool.T + fc_out_b                          # [BL, 3]
        outs.append(out_core)
    full = np.concatenate(outs, axis=0).astype(np.float32)
    if _want_trace:
        _CACHE["last_result"] = res
    return full
